# revision 1
# baseline (speedup 1.0000x reference)
"""BERT self-attention with relative_key_query position scores and per-head
conditional gating, as a Bass/Tile kernel on 8 Trainium2 NeuronCores.

Sharding: data-parallel over batch (B=16 -> 2 per core). Weights replicated.

Per-core pipeline (BL=2 batches, TOK=1024 tokens), bf16 matmul path with
fp32 PSUM accumulation and an fp32 softmax chain:
  hsT   = hs^T (PE transposes, bf16)
  qT/kT = bf16 [dout, tok] layout (heads on partitions); vN = bf16 [tok, dout]
  gateT = sigmoid(hs @ blockdiag(gate_w) + gate_b)  (fp32 out)
  per (b, h):
    A' = q @ Erev^T, Bm = k @ E^T -> bf16 DRAM scratch (width-640 windows)
    S1[l,r] = A'[l, 127-l+r]   read via skewed AP (contiguous rows)
    S2 blocks read via XBAR DMA-transpose of the skewed Bm view
    scores(psum) = q@k^T + S2 blocks + S1, injected via identity matmuls
    probs = Relu(exp(scores/8) * (c/rowsum) + gamma)   [clipped softmax == this]
    ctx   = (probs^T via PE transpose) @ v, scaled by gate
"""

import sys

sys.path.insert(0, "/opt/trn_rl_repo")

import numpy as np

import concourse.bass as bass
import concourse.mybir as mybir
import concourse.tile as tile
from concourse import bacc
from concourse.masks import make_identity

P = 128
B, S, D = 16, 512, 1024
H, DH = 16, 64
NCORES = 8
BL = B // NCORES          # batches per core
TOK = BL * S              # tokens per core
MAXPOS = 512
GAMMA = -12.0 / 512.0     # -0.0234375
CSCALE = 1.0 - GAMMA      # eta - gamma = 1.0234375
JW = 640                  # scratch window width per 128-row chunk
NE = 2 * MAXPOS - 1       # 1023 distance-embedding rows

f32 = mybir.dt.float32
bf16 = mybir.dt.bfloat16
AF = mybir.ActivationFunctionType


def _skew(dtile):
    """[128, JW] dram tile -> [128, 512] diagonal-band view:
    band[i, r] = tile[i, 127 - i + r]; flat AP [[JW-1, 128], [1, 512]]
    at offset 127."""
    flat = dtile.rearrange("p w -> (p w)")
    return flat[127:127 + 128 * (JW - 1)].rearrange("(p x) -> p x", x=JW - 1)[:, :S]


def _skew3(dtile):
    """[4, 128, JW] dram tile -> [128, 4, 512] batched diagonal-band view:
    out[p, c, r] = chunk c's band[p, 127 - p + r]; one DMA for all chunks."""
    v = dtile.rearrange("c p w -> c (p w)")          # [4, P*JW]
    v = v[:, 127:127 + P * (JW - 1)]                 # in-bounds: 127+P*(JW-1) <= P*JW
    v = v.rearrange("c (p x) -> c p x", x=JW - 1)    # [4, 128, 639]
    return v[:, :, :S].rearrange("c p x -> p c x")


def build_program():
    nc = bacc.Bacc(None, target_bir_lowering=False)

    hs = nc.dram_tensor("hs", [TOK, D], f32, kind="ExternalInput")
    Wq = nc.dram_tensor("Wq", [D, D], f32, kind="ExternalInput")
    Wk = nc.dram_tensor("Wk", [D, D], f32, kind="ExternalInput")
    Wv = nc.dram_tensor("Wv", [D, D], f32, kind="ExternalInput")
    bq = nc.dram_tensor("bq", [D], f32, kind="ExternalInput")
    bk = nc.dram_tensor("bk", [D], f32, kind="ExternalInput")
    bv = nc.dram_tensor("bv", [D], f32, kind="ExternalInput")
    emb = nc.dram_tensor("emb", [NE, DH], f32, kind="ExternalInput")
    embr = nc.dram_tensor("embr", [NE, DH], f32, kind="ExternalInput")
    gw = nc.dram_tensor("gw", [H, DH], f32, kind="ExternalInput")
    gb = nc.dram_tensor("gb", [H], f32, kind="ExternalInput")
    out = nc.dram_tensor("out", [TOK, D], f32, kind="ExternalOutput")

    with tile.TileContext(nc) as tc:
        _emit(nc, tc, hs, (Wq, Wk, Wv), (bq, bk, bv), (emb, embr), gw, gb, out)
    nc.compile()
    return nc


def _emit(nc, tc, hs, Ws, bs, embs, gw, gb, out):
    TP = TOK // P    # 8 token blocks of 128
    TB = TOK // 512  # 2 token blocks of 512
    KO = D // P      # 8 contraction blocks

    with (
        tc.tile_pool(name="const", bufs=1) as const,
        tc.tile_pool(name="hsT_p", bufs=1) as hsT_p,
    ):
        ident = const.tile([P, P], f32)
        make_identity(nc, ident[:])
        ident_bf = const.tile([P, P], bf16, tag="identb")
        make_identity(nc, ident_bf[:])
        gamma_col = const.tile([P, 1], f32, tag="gamma")
        nc.gpsimd.memset(gamma_col[:], GAMMA)
        ones_row = const.tile([1, P], f32, tag="ones")
        nc.gpsimd.memset(ones_row[:], 1.0)

        # biases: bq_sb[p, o] = bq[o*128 + p]; bv as a free-dim row
        bq_sb = const.tile([P, KO], f32, tag="bq")
        bk_sb = const.tile([P, KO], f32, tag="bk")
        nc.sync.dma_start(bq_sb[:], bs[0][:].rearrange("(o p) -> p o", p=P))
        nc.sync.dma_start(bk_sb[:], bs[1][:].rearrange("(o p) -> p o", p=P))
        bv_row = const.tile([1, D], f32, tag="bv")
        nc.sync.dma_start(bv_row[:], bs[2][:, None].rearrange("d a -> a d"))
        gb_row = const.tile([1, H], f32, tag="gb")
        nc.sync.dma_start(gb_row[:], gb[:, None].rearrange("d a -> a d"))

        # gate weights as block-diagonal [din(p,o), h], bf16 (cast DMA)
        gw_sb = const.tile([P, KO, H], bf16, tag="gw")
        nc.gpsimd.memset(gw_sb[:], 0.0)
        for h in range(H):
            p0 = 64 * (h % 2)
            nc.gpsimd.dma_start(
                gw_sb[p0:p0 + DH, h // 2, h:h + 1], gw[h, :, None]
            )

        # E^T and Erev^T in bf16, duplicated into both partition halves
        ET = const.tile([P, 1024], bf16, tag="ET")
        ERT = const.tile([P, 1024], bf16, tag="ERT")
        with (
            tc.tile_pool(name="ep", bufs=2) as ep,
            tc.tile_pool(name="epp", bufs=4, space="PSUM") as epp,
        ):
            for dst, rev in ((ET, False), (ERT, True)):
                esb = ep.tile([P, 8, DH], f32, tag="esb")
                nc.gpsimd.memset(esb[:], 0.0)
                src = embs[1][:] if rev else embs[0][:]
                nc.sync.dma_start(
                    esb[:, 0:7, :], src[0:896].rearrange("(o p) d -> p o d", p=P)
                )
                nc.sync.dma_start(esb[0:127, 7, :], src[896:NE])
                for o in range(8):
                    pt = epp.tile([P, P], f32, tag="ept")
                    nc.tensor.transpose(pt[0:DH, :], esb[:, o, :], ident[:])
                    nc.vector.tensor_copy(dst[0:DH, o * P:(o + 1) * P], pt[0:DH, :])
                # duplicate into partitions 64..127 so rhs can match any
                # lhsT head base partition
                nc.sync.dma_start(dst[DH:P, :], dst[0:DH, :])

        # ---- phase A: load hs (cast to bf16) and build hsT [din(p,o), tok]
        hsT = hsT_p.tile([P, KO, TOK], bf16)
        with (
            tc.tile_pool(name="hsp", bufs=1) as hsp,
            tc.tile_pool(name="psA", bufs=4, space="PSUM") as psA,
        ):
            hs_sb = hsp.tile([P, TP, D], bf16)
            hs_r = hs[:].rearrange("(o p) d -> p o d", p=P)
            for to in range(TP):
                nc.gpsimd.dma_start(hs_sb[:, to, :], hs_r[:, to, :])
            for to in range(TP):
                for do in range(KO):
                    pt = psA.tile([P, P], bf16)
                    nc.tensor.transpose(
                        pt[:], hs_sb[:, to, do * P:(do + 1) * P], ident_bf[:]
                    )
                    if (to + do) % 2 == 0:
                        nc.scalar.copy(hsT[:, do, to * P:(to + 1) * P], pt[:])
                    else:
                        nc.vector.tensor_copy(hsT[:, do, to * P:(to + 1) * P], pt[:])

        # ---- phase B: QKV projections + gate (bf16 matmuls, fp32 psum)
        with tc.tile_pool(name="qkv", bufs=1) as qkvp:
            qT = qkvp.tile([P, KO, TOK], bf16, tag="qT")
            kT = qkvp.tile([P, KO, TOK], bf16, tag="kT")
            vN = qkvp.tile([P, TP, D], bf16, tag="vN")
            gateT = qkvp.tile([P, TP, H], f32, tag="gateT")

            with (
                tc.tile_pool(name="wp", bufs=3) as wp,
                tc.tile_pool(name="psB", bufs=4, space="PSUM") as psB,
            ):
                for wi, (W, dst) in enumerate(((Ws[0], qT), (Ws[1], kT))):
                    w_sb = wp.tile([P, KO, D], bf16, tag="w")
                    w_r = W[:].rearrange("(o p) n -> p o n", p=P)
                    for kk in range(0, KO, 2):
                        nc.gpsimd.dma_start(
                            w_sb[:, kk:kk + 2, :], w_r[:, kk:kk + 2, :]
                        )
                    bias = bq_sb if wi == 0 else bk_sb
                    for do in range(KO):
                        for tb in range(TB):
                            ps = psB.tile([P, 512], f32)
                            for kk in range(KO):
                                nc.tensor.matmul(
                                    ps[:],
                                    lhsT=w_sb[:, kk, do * P:(do + 1) * P],
                                    rhs=hsT[:, kk, tb * 512:(tb + 1) * 512],
                                    start=(kk == 0),
                                    stop=(kk == KO - 1),
                                )
                            nc.vector.tensor_scalar_add(
                                dst[:, do, tb * 512:(tb + 1) * 512], ps[:],
                                bias[:, do:do + 1],
                            )

            # ---- phase C: attention, software-pipelined across (b, h) pairs:
            # stage 1 (pos matmuls + scratch roundtrip issue) runs one pair
            # ahead of stage 2 (scores/softmax/pv), so the DRAM latency of
            # pair N hides behind pair N-1's compute.
            with (
                tc.tile_pool(name="vwp", bufs=1) as vwp,
                tc.tile_pool(name="ddr", bufs=10, space="DRAM") as ddr,
                tc.tile_pool(name="ddrB", bufs=8, space="DRAM") as ddrB,
                tc.tile_pool(name="posb", bufs=4) as posb,
                tc.tile_pool(name="posbB", bufs=2) as posbB,
                tc.tile_pool(name="s2tp", bufs=10) as s2tp,
                tc.tile_pool(name="s1p", bufs=5) as s1p,
                tc.tile_pool(name="expp", bufs=8) as expp,
                tc.tile_pool(name="prp", bufs=6) as prp,
                tc.tile_pool(name="ptp", bufs=2) as ptp,
                tc.tile_pool(name="smp", bufs=4) as smp,
                tc.tile_pool(name="outp", bufs=3) as outp,
                tc.tile_pool(name="pp_pos", bufs=2, space="PSUM") as pp_pos,
                tc.tile_pool(name="pp_sc", bufs=2, space="PSUM") as pp_sc,
                tc.tile_pool(name="pp_tp", bufs=1, space="PSUM") as pp_tp,
                tc.tile_pool(name="pp_pv", bufs=1, space="PSUM") as pp_pv,
            ):
                def heads_of(b, h):
                    base = 64 * (h % 2)
                    ho = h // 2
                    return (
                        qT[base:base + DH, ho, b * S:(b + 1) * S],
                        kT[base:base + DH, ho, b * S:(b + 1) * S],
                        base,
                    )

                def emit_pos(b, h):
                    qh, kh, base = heads_of(b, h)
                    scr = []
                    for side, (src, ew) in enumerate(((qh, ERT), (kh, ET))):
                        dt_sb = bf16 if side == 0 else f32
                        pool_ = posb if side == 0 else posbB
                        sb = pool_.tile([P, 4, JW], dt_sb)
                        for c in range(4):
                            jst = 384 - c * 128
                            pp = pp_pos.tile([P, JW], f32, tag="pos")
                            nc.tensor.matmul(
                                pp[:, 0:512],
                                lhsT=src[:, c * P:(c + 1) * P],
                                rhs=ew[base:base + DH, jst:jst + 512],
                                start=True, stop=True,
                            )
                            nc.tensor.matmul(
                                pp[:, 512:JW],
                                lhsT=src[:, c * P:(c + 1) * P],
                                rhs=ew[base:base + DH, jst + 512:jst + JW],
                                start=True, stop=True,
                            )
                            if side == 0 and c == 0:
                                nc.scalar.copy(sb[:, c, :], pp[:])
                            else:
                                nc.vector.tensor_copy(sb[:, c, :], pp[:])
                        dpool = ddr if side == 0 else ddrB
                        dt_ = dpool.tile([4, P, JW], dt_sb)
                        nc.gpsimd.dma_start(
                            dt_[:].rearrange("c p w -> p c w"), sb[:]
                        )
                        scr.append(dt_)

                    # issue the band reads right away so they land during the
                    # previous pair's compute
                    s1t = s1p.tile([P, 4, S], bf16)
                    nc.sync.dma_start(
                        s1t[:],
                        _skew3(scr[0]),
                    )
                    s2 = []
                    for rc in range(4):
                        t2 = s2tp.tile([P, S], f32)
                        nc.sync.dma_start(t2[:], _skew(scr[1][rc]))
                        s2.append(t2)
                    return (s1t, s2)

                def emit_v_gate():
                    w_sb = vwp.tile([P, KO, D], bf16, tag="wv")
                    w_r = Ws[2][:].rearrange("(o p) n -> p o n", p=P)
                    for kk in range(0, KO, 2):
                        nc.gpsimd.dma_start(
                            w_sb[:, kk:kk + 2, :], w_r[:, kk:kk + 2, :]
                        )
                    for to in range(TP):
                        for nb in range(2):
                            ps = pp_sc.tile([P, S], f32, tag="ps")
                            for kk in range(KO):
                                nc.tensor.matmul(
                                    ps[:],
                                    lhsT=hsT[:, kk, to * P:(to + 1) * P],
                                    rhs=w_sb[:, kk, nb * 512:(nb + 1) * 512],
                                    start=(kk == 0),
                                    stop=False,
                                )
                            nc.tensor.matmul(
                                ps[:], lhsT=ones_row[:],
                                rhs=bv_row[0:1, nb * 512:(nb + 1) * 512],
                                start=False, stop=True,
                            )
                            nc.vector.tensor_copy(
                                vN[:, to, nb * 512:(nb + 1) * 512], ps[:]
                            )
                    for to in range(TP):
                        psg = pp_pv.tile([P, DH], f32, tag="pv")
                        for kk in range(KO):
                            nc.tensor.matmul(
                                psg[:, 0:H],
                                lhsT=hsT[:, kk, to * P:(to + 1) * P],
                                rhs=gw_sb[:, kk, :],
                                start=(kk == 0),
                                stop=False,
                            )
                        nc.tensor.matmul(
                            psg[:, 0:H], lhsT=ones_row[:], rhs=gb_row[:],
                            start=False, stop=True,
                        )
                        nc.scalar.activation(gateT[:, to, :], psg[:, 0:H], AF.Sigmoid)

                def emit_attn(b, h, st):
                    qh, kh, base = heads_of(b, h)
                    s1t, s2 = st
                    sums = smp.tile([P, 4], f32, tag="sums")
                    exps = []
                    for lc in range(4):
                        ps = pp_sc.tile([P, S], f32, tag="ps")
                        nc.tensor.matmul(
                            ps[:],
                            lhsT=qh[:, lc * P:(lc + 1) * P],
                            rhs=kh[:],
                            start=True, stop=False,
                        )
                        for rc in range(4):
                            nc.tensor.matmul(
                                ps[:, rc * P:(rc + 1) * P],
                                lhsT=s2[rc][:, lc * P:(lc + 1) * P],
                                rhs=ident[:],
                                is_transpose=True,
                                start=False, stop=False,
                            )
                        nc.tensor.matmul(
                            ps[:], lhsT=ident_bf[:], rhs=s1t[:, lc, :],
                            start=False, stop=True,
                        )
                        ex = expp.tile([P, S], f32)
                        nc.scalar.activation(
                            ex[:], ps[:], AF.Exp, scale=0.125,
                            accum_out=sums[:, lc:lc + 1],
                        )
                        exps.append(ex)

                    inv = smp.tile([P, 4], f32, tag="inv")
                    nc.vector.reciprocal(inv[:], sums[:])
                    nc.vector.tensor_scalar_mul(inv[:], inv[:], CSCALE)

                    # probs (bf16) -> probsT via PE transpose; one batched
                    # psum->sbuf copy per l-chunk
                    pT = ptp.tile([P, 4, S], bf16)
                    for lc in range(4):
                        pr = prp.tile([P, S], bf16)
                        nc.scalar.activation(
                            pr[:], exps[lc][:], AF.Relu,
                            bias=gamma_col[:], scale=inv[:, lc:lc + 1],
                        )
                        tp = pp_tp.tile([P, 4, P], bf16)
                        for rc in range(4):
                            nc.tensor.transpose(
                                tp[:, rc, :], pr[:, rc * P:(rc + 1) * P],
                                ident_bf[:],
                            )
                        if lc % 2 == 0:
                            nc.vector.tensor_copy(
                                pT[:, :, lc * P:(lc + 1) * P], tp[:])
                        else:
                            nc.scalar.copy(
                                pT[:, :, lc * P:(lc + 1) * P], tp[:])

                    # ctx = probs @ v, gated; one batched out DMA per pair
                    ot = outp.tile([P, 4, DH], f32)
                    for lc in range(4):
                        pv = pp_pv.tile([P, DH], f32, tag="pv")
                        for rc in range(4):
                            nc.tensor.matmul(
                                pv[:],
                                lhsT=pT[:, rc, lc * P:(lc + 1) * P],
                                rhs=vN[:, b * 4 + rc, h * DH:(h + 1) * DH],
                                start=(rc == 0), stop=(rc == 3),
                            )
                        nc.vector.tensor_scalar_mul(
                            ot[:, lc, :], pv[:], gateT[:, b * 4 + lc, h:h + 1]
                        )
                    nc.gpsimd.dma_start(
                        out[b * S:(b + 1) * S, h * DH:(h + 1) * DH]
                        .rearrange("(c p) d -> p c d", p=P),
                        ot[:],
                    )

                pairs = [(b, h) for b in range(BL) for h in range(H)]
                from collections import deque
                pending = deque()
                DEPTH = 2
                for i, (b, h) in enumerate(pairs):
                    st = emit_pos(b, h)
                    pending.append((b, h, st))
                    if i == 0:
                        emit_v_gate()
                    if len(pending) > DEPTH:
                        emit_attn(*pending.popleft())
                while pending:
                    emit_attn(*pending.popleft())


_NC_CACHE = {}


def _get_program():
    if "nc" not in _NC_CACHE:
        _NC_CACHE["nc"] = build_program()
    return _NC_CACHE["nc"]


def make_in_maps(inputs):
    hs = np.ascontiguousarray(np.asarray(inputs["hidden_states"], dtype=np.float32))
    maps = []
    shared = {
        "Wq": np.asarray(inputs["Wq"], np.float32),
        "Wk": np.asarray(inputs["Wk"], np.float32),
        "Wv": np.asarray(inputs["Wv"], np.float32),
        "bq": np.asarray(inputs["bq"], np.float32),
        "bk": np.asarray(inputs["bk"], np.float32),
        "bv": np.asarray(inputs["bv"], np.float32),
        "emb": np.asarray(inputs["dist_emb"], np.float32),
        "embr": np.ascontiguousarray(
            np.asarray(inputs["dist_emb"], np.float32)[::-1]
        ),
        "gw": np.asarray(inputs["gate_w"], np.float32),
        "gb": np.asarray(inputs["gate_b"], np.float32),
    }
    for c in range(NCORES):
        m = dict(shared)
        m["hs"] = np.ascontiguousarray(
            hs[c * BL:(c + 1) * BL].reshape(TOK, D)
        )
        maps.append(m)
    return maps


def kernel(**inputs):
    from concourse.bass_utils import run_bass_kernel_spmd

    nc = _get_program()
    in_maps = make_in_maps(inputs)
    res = run_bass_kernel_spmd(nc, in_maps, core_ids=list(range(NCORES)))
    return np.concatenate(
        [res.results[c]["out"].reshape(BL, S, D) for c in range(NCORES)], axis=0
    )



# revision 49
# speedup vs baseline: 1.1800x; 1.1800x over previous
"""BERT self-attention with relative_key_query position scores and per-head
conditional gating, as a Bass/Tile kernel on 8 Trainium2 NeuronCores.

Sharding: data-parallel over batch (B=16 -> 2 per core). Weights replicated.

Per-core pipeline (BL=2 batches, TOK=1024 tokens), bf16 matmul path with
fp32 PSUM accumulation and an fp32 softmax chain:
  hsT   = hs^T (PE transposes, bf16)
  qT/kT = bf16 [dout, tok] layout (heads on partitions); vN = bf16 [tok, dout]
  gateT = sigmoid(hs @ blockdiag(gate_w) + gate_b)  (fp32 out)
  per (b, h):
    A' = q @ Erev^T, Bm = k @ E^T -> bf16 DRAM scratch (width-640 windows)
    S1[l,r] = A'[l, 127-l+r]   read via skewed AP (contiguous rows)
    S2 blocks read via XBAR DMA-transpose of the skewed Bm view
    scores(psum) = q@k^T + S2 blocks + S1, injected via identity matmuls
    probs = Relu(exp(scores/8) * (c/rowsum) + gamma)   [clipped softmax == this]
    ctx   = (probs^T via PE transpose) @ v, scaled by gate
"""

import sys

sys.path.insert(0, "/opt/trn_rl_repo")

import numpy as np

import concourse.bass as bass
import concourse.mybir as mybir
import concourse.tile as tile
from concourse import bacc
from concourse.masks import make_identity

P = 128
B, S, D = 16, 512, 1024
H, DH = 16, 64
NCORES = 8
BL = B // NCORES          # batches per core
TOK = BL * S              # tokens per core
MAXPOS = 512
GAMMA = -12.0 / 512.0     # -0.0234375
CSCALE = 1.0 - GAMMA      # eta - gamma = 1.0234375
GOC = -GAMMA / CSCALE     # |gamma| / c
JW = 640                  # scratch window width per 128-row chunk
NE = 2 * MAXPOS - 1       # 1023 distance-embedding rows

f32 = mybir.dt.float32
bf16 = mybir.dt.bfloat16
f8 = mybir.dt.float8e4
AF = mybir.ActivationFunctionType


def _skew(dtile):
    """[128, JW] dram tile -> [128, 512] diagonal-band view:
    band[i, r] = tile[i, 127 - i + r]; flat AP [[JW-1, 128], [1, 512]]
    at offset 127."""
    flat = dtile.rearrange("p w -> (p w)")
    return flat[127:127 + 128 * (JW - 1)].rearrange("(p x) -> p x", x=JW - 1)[:, :S]


def _skew3(dtile):
    """[4, 128, JW] dram tile -> [128, 4, 512] batched diagonal-band view:
    out[p, c, r] = chunk c's band[p, 127 - p + r]; one DMA for all chunks."""
    v = dtile.rearrange("c p w -> c (p w)")          # [4, P*JW]
    v = v[:, 127:127 + P * (JW - 1)]                 # in-bounds: 127+P*(JW-1) <= P*JW
    v = v.rearrange("c (p x) -> c p x", x=JW - 1)    # [4, 128, 639]
    return v[:, :, :S].rearrange("c p x -> p c x")


def build_program():
    nc = bacc.Bacc(None, target_bir_lowering=False)

    hs = nc.dram_tensor("hs", [TOK, D], f32, kind="ExternalInput")
    Wq = nc.dram_tensor("Wq", [D, D], f32, kind="ExternalInput")
    Wk = nc.dram_tensor("Wk", [D, D], f32, kind="ExternalInput")
    Wv = nc.dram_tensor("Wv", [D, D], f32, kind="ExternalInput")
    bq = nc.dram_tensor("bq", [D], f32, kind="ExternalInput")
    bk = nc.dram_tensor("bk", [D], f32, kind="ExternalInput")
    bv = nc.dram_tensor("bv", [D], f32, kind="ExternalInput")
    emb = nc.dram_tensor("emb", [NE, DH], f32, kind="ExternalInput")
    embr = nc.dram_tensor("embr", [NE, DH], f32, kind="ExternalInput")
    gw = nc.dram_tensor("gw", [H, DH], f32, kind="ExternalInput")
    gb = nc.dram_tensor("gb", [H], f32, kind="ExternalInput")
    out = nc.dram_tensor("out", [TOK, D], f32, kind="ExternalOutput")

    with tile.TileContext(nc) as tc:
        _emit(nc, tc, hs, (Wq, Wk, Wv), (bq, bk, bv), (emb, embr), gw, gb, out)
    nc.compile()
    return nc


def _emit(nc, tc, hs, Ws, bs, embs, gw, gb, out):
    TP = TOK // P    # 8 token blocks of 128
    TB = TOK // 512  # 2 token blocks of 512
    KO = D // P      # 8 contraction blocks

    with (
        tc.tile_pool(name="const", bufs=1) as const,
        tc.tile_pool(name="hsT_p", bufs=1) as hsT_p,
    ):
        ident = const.tile([P, P], f32)
        make_identity(nc, ident[:])
        ident_bf = const.tile([P, P], bf16, tag="identb")
        make_identity(nc, ident_bf[:])
        ident8 = const.tile([P, P], f8, tag="ident8")
        make_identity(nc, ident8[:])
        gamma_col = const.tile([P, 1], f32, tag="gamma")
        nc.gpsimd.memset(gamma_col[:], GAMMA)
        ones_row = const.tile([1, P], f32, tag="ones")
        nc.gpsimd.memset(ones_row[:], 1.0)

        # biases: bq_sb[p, o] = bq[o*128 + p]; bv as a free-dim row
        bq_sb = const.tile([P, KO], f32, tag="bq")
        bk_sb = const.tile([P, KO], f32, tag="bk")
        nc.sync.dma_start(bq_sb[:], bs[0][:].rearrange("(o p) -> p o", p=P))
        nc.sync.dma_start(bk_sb[:], bs[1][:].rearrange("(o p) -> p o", p=P))
        bv_row = const.tile([1, D], f32, tag="bv")
        nc.sync.dma_start(bv_row[:], bs[2][:, None].rearrange("d a -> a d"))
        gb_row = const.tile([1, H], f32, tag="gb")
        nc.sync.dma_start(gb_row[:], gb[:, None].rearrange("d a -> a d"))

        # gate weights as block-diagonal [din(p,o), h], bf16 (cast DMA);
        # filled lazily inside emit_v_gate so the startup DMA queue stays
        # short.
        gw_sb = const.tile([P, KO, H], bf16, tag="gw")

        def build_gw():
            nc.gpsimd.memset(gw_sb[:], 0.0)
            for h in range(H):
                p0 = 64 * (h % 2)
                nc.gpsimd.dma_start(
                    gw_sb[p0:p0 + DH, h // 2, h:h + 1], gw[h, :, None]
                )

        # E^T and Erev^T in f8 with a zero slot: rhs [d, 2, n] for the
        # DoubleRow pos matmuls (lhsT broadcasts q twice; slot 1 adds q.0=0).
        # Duplicated into both partition halves to match any head base.
        ET8 = const.tile([P, 2, 1024], f8, tag="ET8")
        ERT8 = const.tile([P, 2, 1024], f8, tag="ERT8")
        with (
            tc.tile_pool(name="ep", bufs=2) as ep,
            tc.tile_pool(name="epp", bufs=4, space="PSUM") as epp,
        ):
            for dst8, rev in ((ET8, False), (ERT8, True)):
                esb = ep.tile([P, 8, DH], f32, tag="esb")
                nc.gpsimd.memset(esb[:], 0.0)
                src = embs[1][:] if rev else embs[0][:]
                nc.sync.dma_start(
                    esb[:, 0:7, :], src[0:896].rearrange("(o p) d -> p o d", p=P)
                )
                nc.sync.dma_start(esb[0:127, 7, :], src[896:NE])
                nc.gpsimd.memset(dst8[0:DH, 1, :], 0.0)
                for o in range(8):
                    pt = epp.tile([P, P], f32, tag="ept")
                    nc.tensor.transpose(pt[0:DH, :], esb[:, o, :], ident[:])
                    nc.vector.tensor_copy(
                        dst8[0:DH, 0, o * P:(o + 1) * P], pt[0:DH, :]
                    )
                nc.sync.dma_start(dst8[DH:P, :, :], dst8[0:DH, :, :])

        # ---- phase A: load hs (cast to bf16) and build hsT [din(p,o), tok]
        hsT = hsT_p.tile([P, KO, TOK], bf16)
        with (
            tc.tile_pool(name="hsp", bufs=2) as hsp,
            tc.tile_pool(name="psA", bufs=4, space="PSUM") as psA,
        ):
            hs_sb = hsp.tile([P, TP, D], bf16)
            hs_r = hs[:].rearrange("(o p) d -> p o d", p=P)
            # split the load: even chunks cast-DMA on Pool, odd chunks raw
            # fp32 on SP + engine cast, so neither DMA queue gates startup
            for to in range(0, TP, 2):
                nc.gpsimd.dma_start(hs_sb[:, to, :], hs_r[:, to, :])
            for to in range(1, TP, 2):
                stg = hsp.tile([P, D], f32, tag="stg")
                nc.sync.dma_start(stg[:], hs_r[:, to, :])
                if to % 4 == 1:
                    nc.vector.tensor_copy(hs_sb[:, to, :], stg[:])
                else:
                    nc.scalar.copy(hs_sb[:, to, :], stg[:])
            for to in range(TP):
                for do in range(KO):
                    pt = psA.tile([P, P], bf16)
                    nc.tensor.transpose(
                        pt[:], hs_sb[:, to, do * P:(do + 1) * P], ident_bf[:]
                    )
                    if (to + do) % 2 == 0:
                        nc.scalar.copy(hsT[:, do, to * P:(to + 1) * P], pt[:])
                    else:
                        nc.vector.tensor_copy(hsT[:, do, to * P:(to + 1) * P], pt[:])

        # ---- phase B+C fused: q/k projection chunks interleave with the
        # attention pair pipeline (each dout chunk unlocks 2 heads), so the
        # projection matmuls act as PE filler while softmax/copy engines
        # drain earlier pairs.
        with tc.tile_pool(name="qkv", bufs=1) as qkvp:
            qT = qkvp.tile([P, KO, TOK], bf16, tag="qT")
            kT = qkvp.tile([P, KO, TOK], bf16, tag="kT")
            q8 = qkvp.tile([P, KO, TOK], f8, tag="q8")
            k8 = qkvp.tile([P, KO, TOK], f8, tag="k8")
            vN = qkvp.tile([P, TP, D], bf16, tag="vN")
            gateT = qkvp.tile([P, TP, H], f32, tag="gateT")

            NB = 3  # band double-buffer depth
            with (
                tc.tile_pool(name="wp", bufs=3) as wp,
                tc.tile_pool(name="vwp", bufs=1) as vwp,
                tc.tile_pool(name="ddr", bufs=10, space="DRAM") as ddr,
                tc.tile_pool(name="ddrB", bufs=8, space="DRAM") as ddrB,
                tc.tile_pool(name="posb", bufs=3) as posb,
                tc.tile_pool(name="posbB", bufs=3) as posbB,
                tc.tile_pool(name="bandp", bufs=1) as bandp,
                tc.tile_pool(name="expp", bufs=6) as expp,
                tc.tile_pool(name="prp", bufs=4) as prp,
                tc.tile_pool(name="ptp", bufs=2) as ptp,
                tc.tile_pool(name="smp", bufs=4) as smp,
                tc.tile_pool(name="outp", bufs=2) as outp,
                tc.tile_pool(name="pp_pos", bufs=2, space="PSUM") as pp_pos,
                tc.tile_pool(name="pp_sc", bufs=2, space="PSUM") as pp_sc,
                tc.tile_pool(name="pp_tp", bufs=1, space="PSUM") as pp_tp,
                tc.tile_pool(name="pp_pv", bufs=1, space="PSUM") as pp_pv,
            ):
                # Band tiles interleave the DMA'd band (slot i) with a
                # constant [I I I I] pattern (slot 1-i) so one fp8 DoubleRow
                # matmul per 128x128 block injects s1 + s2^T:
                #   psum += lhsT[:,0].T@rhs[:,0] + lhsT[:,1].T@rhs[:,1]
                #         = I.T@s1 + s2.T@I
                s1c = bandp.tile([P, NB, 2, 4, S], f8, tag="s1c")
                s2c = bandp.tile([P, NB, 2, 4, S], f8, tag="s2c")
                i4 = bandp.tile([P, 4, S], f8, tag="i4")

                def build_bands_const():
                    nc.gpsimd.dma_start(i4[:, 0, 0:P], ident8[:])
                    nc.gpsimd.dma_start(i4[:, 0, P:2 * P], i4[:, 0, 0:P])
                    nc.gpsimd.dma_start(i4[:, 0, 2 * P:S], i4[:, 0, 0:2 * P])
                    nc.gpsimd.dma_start(i4[:, 1:2, :], i4[:, 0:1, :])
                    nc.gpsimd.dma_start(i4[:, 2:4, :], i4[:, 0:2, :])
                    for n in range(NB):
                        nc.gpsimd.dma_start(s1c[:, n, 1, :, :], i4[:])
                        nc.gpsimd.dma_start(s2c[:, n, 0, :, :], i4[:])

                def heads_of(b, h):
                    base = 64 * (h % 2)
                    ho = h // 2
                    return (
                        qT[base:base + DH, ho, b * S:(b + 1) * S],
                        kT[base:base + DH, ho, b * S:(b + 1) * S],
                        base,
                    )

                def heads8_of(b, h):
                    base = 64 * (h % 2)
                    ho = h // 2
                    return (
                        q8[base:base + DH, ho, b * S:(b + 1) * S],
                        k8[base:base + DH, ho, b * S:(b + 1) * S],
                        base,
                    )

                # W chunks loaded on demand (dout slice do), prefetched one
                # chunk ahead so the in-order PE queue never waits on them.
                w_r_q = Ws[0][:].rearrange("(o p) n -> p o n", p=P)
                w_r_k = Ws[1][:].rearrange("(o p) n -> p o n", p=P)
                w_tiles = {}

                def load_w_chunk(do):
                    for tag, w_r_ in (("wq", w_r_q), ("wk", w_r_k)):
                        wt = wp.tile([P, KO, P], bf16, tag=tag)
                        nc.gpsimd.dma_start(
                            wt[:], w_r_[:, :, do * P:(do + 1) * P]
                        )
                        w_tiles[(tag, do)] = wt

                load_w_chunk(0)

                def emit_qk_chunk(do):
                    if do + 1 < KO:
                        load_w_chunk(do + 1)
                    for tag, dst, dst8, bias in (
                        ("wq", qT, q8, bq_sb),
                        ("wk", kT, k8, bk_sb),
                    ):
                        w_sb = w_tiles.pop((tag, do))
                        for tb in range(TB):
                            ps = pp_pos.tile([P, JW], f32, tag="pos")
                            for kk in range(KO):
                                nc.tensor.matmul(
                                    ps[:, 0:512],
                                    lhsT=w_sb[:, kk, :],
                                    rhs=hsT[:, kk, tb * 512:(tb + 1) * 512],
                                    start=(kk == 0),
                                    stop=(kk == KO - 1),
                                )
                            nc.vector.tensor_scalar_add(
                                dst[:, do, tb * 512:(tb + 1) * 512],
                                ps[:, 0:512], bias[:, do:do + 1],
                            )
                            nc.gpsimd.tensor_copy(
                                dst8[:, do, tb * 512:(tb + 1) * 512],
                                dst[:, do, tb * 512:(tb + 1) * 512],
                            )

                def emit_pos(b, h, n):
                    qh8, kh8, base = heads8_of(b, h)
                    scr = []
                    for side, (src, ew) in enumerate(((qh8, ERT8), (kh8, ET8))):
                        pool_ = posb if side == 0 else posbB
                        sb = pool_.tile([P, 4, JW], f8)
                        for c in range(4):
                            jst = 384 - c * 128
                            lhs2 = src[:, c * P:(c + 1) * P].unsqueeze(
                                1).broadcast_to([DH, 2, P])
                            pp = pp_pos.tile([P, JW], f32, tag="pos")
                            nc.tensor.matmul(
                                pp[:, 0:512],
                                lhsT=lhs2,
                                rhs=ew[base:base + DH, :, jst:jst + 512],
                                perf_mode=mybir.MatmulPerfMode.DoubleRow,
                                start=True, stop=True,
                            )
                            nc.tensor.matmul(
                                pp[:, 512:JW],
                                lhsT=lhs2,
                                rhs=ew[base:base + DH, :, jst + 512:jst + JW],
                                perf_mode=mybir.MatmulPerfMode.DoubleRow,
                                start=True, stop=True,
                            )
                            if c % 2 == 0:
                                nc.scalar.copy(sb[:, c, :], pp[:])
                            else:
                                nc.vector.tensor_copy(sb[:, c, :], pp[:])
                        dpool = ddr if side == 0 else ddrB
                        dt_ = dpool.tile([4, P, JW], f8)
                        nc.sync.dma_start(
                            dt_[:].rearrange("c p w -> p c w"), sb[:]
                        )
                        scr.append(dt_)

                    # issue the band reads right away so they land during the
                    # previous pair's compute
                    nc.sync.dma_start(s1c[:, n, 0, :, :], _skew3(scr[0]))
                    nc.sync.dma_start(s2c[:, n, 1, :, :], _skew3(scr[1]))

                vw_sb = vwp.tile([P, KO, D], bf16, tag="wv")

                def emit_v_gate(half):
                    w_sb = vw_sb
                    if half == 0:
                        build_gw()
                        w_r = Ws[2][:].rearrange("(o p) n -> p o n", p=P)
                        for kk in range(0, KO, 2):
                            nc.gpsimd.dma_start(
                                w_sb[:, kk:kk + 2, :], w_r[:, kk:kk + 2, :]
                            )
                    for to in range(4 * half, 4 * half + 4):
                        for nb in range(2):
                            ps = pp_sc.tile([P, S], f32, tag="ps")
                            for kk in range(KO):
                                nc.tensor.matmul(
                                    ps[:],
                                    lhsT=hsT[:, kk, to * P:(to + 1) * P],
                                    rhs=w_sb[:, kk, nb * 512:(nb + 1) * 512],
                                    start=(kk == 0),
                                    stop=False,
                                )
                            nc.tensor.matmul(
                                ps[:], lhsT=ones_row[:],
                                rhs=bv_row[0:1, nb * 512:(nb + 1) * 512],
                                start=False, stop=True,
                            )
                            # fold the clipped-softmax scale c into v
                            nc.vector.tensor_scalar_mul(
                                vN[:, to, nb * 512:(nb + 1) * 512], ps[:], CSCALE
                            )
                    for to in range(4 * half, 4 * half + 4):
                        psg = pp_pv.tile([P, DH], f32, tag="pv")
                        for kk in range(KO):
                            nc.tensor.matmul(
                                psg[:, 0:H],
                                lhsT=hsT[:, kk, to * P:(to + 1) * P],
                                rhs=gw_sb[:, kk, :],
                                start=(kk == 0),
                                stop=False,
                            )
                        nc.tensor.matmul(
                            psg[:, 0:H], lhsT=ones_row[:], rhs=gb_row[:],
                            start=False, stop=True,
                        )
                        nc.scalar.activation(gateT[:, to, :], psg[:, 0:H], AF.Sigmoid)

                def emit_attn(b, h, n):
                    qh, kh, base = heads_of(b, h)
                    sums = smp.tile([P, 4], f32, tag="sums")
                    exps = []
                    for lc in range(4):
                        ps = pp_sc.tile([P, S], f32, tag="ps")
                        nc.tensor.matmul(
                            ps[:],
                            lhsT=qh[:, lc * P:(lc + 1) * P],
                            rhs=kh[:],
                            start=True, stop=False,
                        )
                        for rc in range(4):
                            nc.tensor.matmul(
                                ps[:, rc * P:(rc + 1) * P],
                                lhsT=s2c[:, n, :, rc, lc * P:(lc + 1) * P],
                                rhs=s1c[:, n, :, lc, rc * P:(rc + 1) * P],
                                perf_mode=mybir.MatmulPerfMode.DoubleRow,
                                start=False, stop=(rc == 3),
                            )
                        ex = expp.tile([P, S], bf16)
                        nc.scalar.activation(
                            ex[:], ps[:], AF.Exp, scale=0.125,
                            accum_out=sums[:, lc:lc + 1],
                        )
                        exps.append(ex)

                    # probs = c*softmax + gamma clipped to [0,1]
                    #       = (c/sums) * max(ex - th, 0),  th = |gamma|*sums/c
                    # c is folded into vN, (1/sums)*gate into the ctx scale.
                    nth = smp.tile([P, 4], f32, tag="nth")
                    nc.vector.tensor_scalar_mul(nth[:], sums[:], -GOC)
                    inv = smp.tile([P, 4], f32, tag="inv")
                    nc.vector.reciprocal(inv[:], sums[:])
                    gs = smp.tile([P, 4], f32, tag="gs")
                    nc.vector.tensor_tensor(
                        gs[:], inv[:],
                        gateT[:, b * 4:b * 4 + 4, h:h + 1]
                        .rearrange("p a o -> p (a o)"),
                        mybir.AluOpType.mult,
                    )

                    # probs (bf16) -> probsT via PE transpose; one batched
                    # psum->sbuf copy per l-chunk
                    pT = ptp.tile([P, 4, S], bf16)
                    for lc in range(4):
                        pr = prp.tile([P, S], bf16)
                        reng = nc.vector if lc % 2 == 0 else nc.gpsimd
                        reng.tensor_scalar(
                            pr[:], exps[lc][:], nth[:, lc:lc + 1], 0.0,
                            op0=mybir.AluOpType.add,
                            op1=mybir.AluOpType.max,
                        )
                        tp = pp_tp.tile([P, 4, P], bf16)
                        for rc in range(4):
                            nc.tensor.transpose(
                                tp[:, rc, :], pr[:, rc * P:(rc + 1) * P],
                                ident_bf[:],
                            )
                        nc.vector.tensor_copy(
                            pT[:, :, lc * P:(lc + 1) * P], tp[:])

                    # ctx = probs @ v, gated; one batched out DMA per pair
                    ot = outp.tile([P, 4, DH], f32)
                    for lc in range(4):
                        pv = pp_pv.tile([P, DH], f32, tag="pv")
                        for rc in range(4):
                            nc.tensor.matmul(
                                pv[:],
                                lhsT=pT[:, rc, lc * P:(lc + 1) * P],
                                rhs=vN[:, b * 4 + rc, h * DH:(h + 1) * DH],
                                start=(rc == 0), stop=(rc == 3),
                            )
                        nc.vector.tensor_scalar_mul(
                            ot[:, lc, :], pv[:], gs[:, lc:lc + 1]
                        )
                    nc.gpsimd.dma_start(
                        out[b * S:(b + 1) * S, h * DH:(h + 1) * DH]
                        .rearrange("(c p) d -> p c d", p=P),
                        ot[:],
                    )

                from collections import deque
                pending = deque()
                DEPTH = 2
                seq = []
                for do in range(KO):
                    seq.append(("qk", do))
                    if do == 0:
                        seq.append(("vg", 0))
                        seq.append(("vg", 1))
                    seq += [("pair", (b, 2 * do + dh))
                            for dh in range(2) for b in range(BL)]
                i = 0
                for kind, arg in seq:
                    if kind == "qk":
                        emit_qk_chunk(arg)
                    elif kind == "vg":
                        emit_v_gate(arg)
                    else:
                        b, h = arg
                        emit_pos(b, h, i % NB)
                        if i == 0:
                            build_bands_const()
                        pending.append((b, h, i % NB))
                        if len(pending) > DEPTH:
                            emit_attn(*pending.popleft())
                        i += 1
                while pending:
                    emit_attn(*pending.popleft())


_NC_CACHE = {}


def _get_program():
    if "nc" not in _NC_CACHE:
        _NC_CACHE["nc"] = build_program()
    return _NC_CACHE["nc"]


def make_in_maps(inputs):
    hs = np.ascontiguousarray(np.asarray(inputs["hidden_states"], dtype=np.float32))
    maps = []
    shared = {
        "Wq": np.asarray(inputs["Wq"], np.float32),
        "Wk": np.asarray(inputs["Wk"], np.float32),
        "Wv": np.asarray(inputs["Wv"], np.float32),
        "bq": np.asarray(inputs["bq"], np.float32),
        "bk": np.asarray(inputs["bk"], np.float32),
        "bv": np.asarray(inputs["bv"], np.float32),
        "emb": np.asarray(inputs["dist_emb"], np.float32),
        "embr": np.ascontiguousarray(
            np.asarray(inputs["dist_emb"], np.float32)[::-1]
        ),
        "gw": np.asarray(inputs["gate_w"], np.float32),
        "gb": np.asarray(inputs["gate_b"], np.float32),
    }
    for c in range(NCORES):
        m = dict(shared)
        m["hs"] = np.ascontiguousarray(
            hs[c * BL:(c + 1) * BL].reshape(TOK, D)
        )
        maps.append(m)
    return maps


def kernel(**inputs):
    from concourse.bass_utils import run_bass_kernel_spmd

    nc = _get_program()
    in_maps = make_in_maps(inputs)
    res = run_bass_kernel_spmd(nc, in_maps, core_ids=list(range(NCORES)))
    return np.concatenate(
        [res.results[c]["out"].reshape(BL, S, D) for c in range(NCORES)], axis=0
    )



# revision 86
# speedup vs baseline: 1.3290x; 1.1263x over previous
"""BERT self-attention with relative_key_query position scores and per-head
conditional gating, as a Bass/Tile kernel on 8 Trainium2 NeuronCores.

Sharding: data-parallel over batch (B=16 -> 2 per core). Weights replicated.

Per-core pipeline (BL=2 batches, TOK=1024 tokens), bf16 matmul path with
fp32 PSUM accumulation and an fp32 softmax chain:
  hsT   = hs^T (PE transposes, bf16)
  qT/kT = bf16 [dout, tok] layout (heads on partitions); vN = bf16 [tok, dout]
  gateT = sigmoid(hs @ blockdiag(gate_w) + gate_b)  (fp32 out)
  per (b, h):
    A' = q @ Erev^T, Bm = k @ E^T -> bf16 DRAM scratch (width-640 windows)
    S1[l,r] = A'[l, 127-l+r]   read via skewed AP (contiguous rows)
    S2 blocks read via XBAR DMA-transpose of the skewed Bm view
    scores(psum) = q@k^T + S2 blocks + S1, injected via identity matmuls
    probs = Relu(exp(scores/8) * (c/rowsum) + gamma)   [clipped softmax == this]
    ctx   = (probs^T via PE transpose) @ v, scaled by gate
"""

import sys

sys.path.insert(0, "/opt/trn_rl_repo")

import numpy as np

import concourse.bass as bass
import concourse.mybir as mybir
import concourse.tile as tile
from concourse import bacc
from concourse.masks import make_identity

P = 128
B, S, D = 16, 512, 1024
H, DH = 16, 64
NCORES = 8
BL = B // NCORES          # batches per core
TOK = BL * S              # tokens per core
MAXPOS = 512
GAMMA = -12.0 / 512.0     # -0.0234375
CSCALE = 1.0 - GAMMA      # eta - gamma = 1.0234375
GOC = -GAMMA / CSCALE     # |gamma| / c
JW = 640                  # scratch window width per 128-row chunk
NE = 2 * MAXPOS - 1       # 1023 distance-embedding rows

f32 = mybir.dt.float32
bf16 = mybir.dt.bfloat16
f8 = mybir.dt.float8e4
AF = mybir.ActivationFunctionType


def _skew(dtile):
    """[128, JW] dram tile -> [128, 512] diagonal-band view:
    band[i, r] = tile[i, 127 - i + r]; flat AP [[JW-1, 128], [1, 512]]
    at offset 127."""
    flat = dtile.rearrange("p w -> (p w)")
    return flat[127:127 + 128 * (JW - 1)].rearrange("(p x) -> p x", x=JW - 1)[:, :S]


def _skew3(dtile):
    """[4, 128, JW] dram tile -> [128, 4, 512] batched diagonal-band view:
    out[p, c, r] = chunk c's band[p, 127 - p + r]; one DMA for all chunks."""
    v = dtile.rearrange("c p w -> c (p w)")          # [4, P*JW]
    v = v[:, 127:127 + P * (JW - 1)]                 # in-bounds: 127+P*(JW-1) <= P*JW
    v = v.rearrange("c (p x) -> c p x", x=JW - 1)    # [4, 128, 639]
    return v[:, :, :S].rearrange("c p x -> p c x")


def build_program():
    nc = bacc.Bacc(None, target_bir_lowering=False)

    hs = nc.dram_tensor("hs", [TOK, D], f32, kind="ExternalInput")
    Wq = nc.dram_tensor("Wq", [D, D], f32, kind="ExternalInput")
    Wk = nc.dram_tensor("Wk", [D, D], f32, kind="ExternalInput")
    Wv = nc.dram_tensor("Wv", [D, D], f32, kind="ExternalInput")
    bq = nc.dram_tensor("bq", [D], f32, kind="ExternalInput")
    bk = nc.dram_tensor("bk", [D], f32, kind="ExternalInput")
    bv = nc.dram_tensor("bv", [D], f32, kind="ExternalInput")
    emb = nc.dram_tensor("emb", [NE, DH], f32, kind="ExternalInput")
    embr = nc.dram_tensor("embr", [NE, DH], f32, kind="ExternalInput")
    gw = nc.dram_tensor("gw", [H, DH], f32, kind="ExternalInput")
    gb = nc.dram_tensor("gb", [H], f32, kind="ExternalInput")
    out = nc.dram_tensor("out", [TOK, D], f32, kind="ExternalOutput")

    with tile.TileContext(nc) as tc:
        _emit(nc, tc, hs, (Wq, Wk, Wv), (bq, bk, bv), (emb, embr), gw, gb, out)
    nc.compile()
    return nc


def _emit(nc, tc, hs, Ws, bs, embs, gw, gb, out):
    TP = TOK // P    # 8 token blocks of 128
    TB = TOK // 512  # 2 token blocks of 512
    KO = D // P      # 8 contraction blocks

    with (
        tc.tile_pool(name="const", bufs=1) as const,
        tc.tile_pool(name="hsT_p", bufs=1) as hsT_p,
    ):
        ident = const.tile([P, P], f32)
        make_identity(nc, ident[:])
        ident_bf = const.tile([P, P], bf16, tag="identb")
        make_identity(nc, ident_bf[:])
        ident8 = const.tile([P, P], f8, tag="ident8")
        make_identity(nc, ident8[:])
        gamma_col = const.tile([P, 1], f32, tag="gamma")
        nc.gpsimd.memset(gamma_col[:], GAMMA)
        ones_row = const.tile([1, P], f32, tag="ones")
        nc.gpsimd.memset(ones_row[:], 1.0)

        # biases: bq_sb[p, o] = bq[o*128 + p]; bv as a free-dim row
        bq_sb = const.tile([P, KO], f32, tag="bq")
        bk_sb = const.tile([P, KO], f32, tag="bk")
        nc.sync.dma_start(bq_sb[:], bs[0][:].rearrange("(o p) -> p o", p=P))
        nc.sync.dma_start(bk_sb[:], bs[1][:].rearrange("(o p) -> p o", p=P))
        bv_row = const.tile([1, D], f32, tag="bv")
        nc.sync.dma_start(bv_row[:], bs[2][:, None].rearrange("d a -> a d"))
        gb_row = const.tile([1, H], f32, tag="gb")
        nc.sync.dma_start(gb_row[:], gb[:, None].rearrange("d a -> a d"))

        # gate weights as block-diagonal [din(p,o), h], bf16 (cast DMA);
        # filled lazily inside emit_v_gate so the startup DMA queue stays
        # short.
        gw_sb = const.tile([P, KO, H], bf16, tag="gw")

        def build_gw():
            nc.gpsimd.memset(gw_sb[:], 0.0)
            for h in range(H):
                p0 = 64 * (h % 2)
                nc.gpsimd.dma_start(
                    gw_sb[p0:p0 + DH, h // 2, h:h + 1], gw[h, :, None]
                )

        # E^T and Erev^T in f8 with a zero slot: rhs [d, 2, n] for the
        # DoubleRow pos matmuls (lhsT broadcasts q twice; slot 1 adds q.0=0).
        # Duplicated into both partition halves to match any head base.
        ET8 = const.tile([P, 2, 1024], f8, tag="ET8")
        ERT8 = const.tile([P, 2, 1024], f8, tag="ERT8")
        with (
            tc.tile_pool(name="ep", bufs=2) as ep,
            tc.tile_pool(name="epp", bufs=4, space="PSUM") as epp,
        ):
            for dst8, rev in ((ET8, False), (ERT8, True)):
                esb = ep.tile([P, 8, DH], f32, tag="esb")
                nc.gpsimd.memset(esb[:], 0.0)
                src = embs[1][:] if rev else embs[0][:]
                nc.sync.dma_start(
                    esb[:, 0:7, :], src[0:896].rearrange("(o p) d -> p o d", p=P)
                )
                nc.sync.dma_start(esb[0:127, 7, :], src[896:NE])
                nc.gpsimd.memset(dst8[0:DH, 1, :], 0.0)
                for o in range(8):
                    pt = epp.tile([P, P], f32, tag="ept")
                    nc.tensor.transpose(pt[0:DH, :], esb[:, o, :], ident[:])
                    nc.vector.tensor_copy(
                        dst8[0:DH, 0, o * P:(o + 1) * P], pt[0:DH, :]
                    )
                nc.sync.dma_start(dst8[DH:P, :, :], dst8[0:DH, :, :])

        # ---- phase A: load hs (cast to bf16) and build hsT [din(p,o), tok]
        hsT = hsT_p.tile([P, KO, TOK], bf16)
        with (
            tc.tile_pool(name="hsp", bufs=2) as hsp,
            tc.tile_pool(name="psA", bufs=4, space="PSUM") as psA,
        ):
            hs_sb = hsp.tile([P, TP, D], bf16)
            hs_r = hs[:].rearrange("(o p) d -> p o d", p=P)
            # split the load: even chunks cast-DMA on Pool, odd chunks raw
            # fp32 on SP + engine cast, so neither DMA queue gates startup
            for to in range(0, TP, 2):
                nc.gpsimd.dma_start(hs_sb[:, to, :], hs_r[:, to, :])
            for to in range(1, TP, 2):
                stg = hsp.tile([P, D], f32, tag="stg")
                nc.sync.dma_start(stg[:], hs_r[:, to, :])
                if to % 4 == 1:
                    nc.vector.tensor_copy(hs_sb[:, to, :], stg[:])
                else:
                    nc.scalar.copy(hs_sb[:, to, :], stg[:])
            for to in range(TP):
                for dg in range(2):
                    pt = psA.tile([P, 4, P], bf16)
                    for dd in range(4):
                        do = 4 * dg + dd
                        nc.tensor.transpose(
                            pt[:, dd, :], hs_sb[:, to, do * P:(do + 1) * P],
                            ident_bf[:],
                        )
                    if (to + dg) % 2 == 0:
                        nc.scalar.copy(
                            hsT[:, 4 * dg:4 * dg + 4, to * P:(to + 1) * P],
                            pt[:],
                        )
                    else:
                        nc.vector.tensor_copy(
                            hsT[:, 4 * dg:4 * dg + 4, to * P:(to + 1) * P],
                            pt[:],
                        )

        # ---- phase B+C fused: q/k projection chunks interleave with the
        # attention pair pipeline (each dout chunk unlocks 2 heads), so the
        # projection matmuls act as PE filler while softmax/copy engines
        # drain earlier pairs.
        with tc.tile_pool(name="qkv", bufs=1) as qkvp:
            qT = qkvp.tile([P, KO, TOK], bf16, tag="qT")
            kT = qkvp.tile([P, KO, TOK], bf16, tag="kT")
            q8 = qkvp.tile([P, KO, TOK], f8, tag="q8")
            k8 = qkvp.tile([P, KO, TOK], f8, tag="k8")
            vN = qkvp.tile([P, TP, D], bf16, tag="vN")
            gateT = qkvp.tile([P, TP, H], f32, tag="gateT")

            NB = 4  # band double-buffer depth
            with (
                tc.tile_pool(name="wp", bufs=3) as wp,
                tc.tile_pool(name="vwp", bufs=1) as vwp,
                tc.tile_pool(name="ddr", bufs=10, space="DRAM") as ddr,
                tc.tile_pool(name="ddrB", bufs=8, space="DRAM") as ddrB,
                tc.tile_pool(name="posb", bufs=3) as posb,
                tc.tile_pool(name="posbB", bufs=3) as posbB,
                tc.tile_pool(name="bandp", bufs=1) as bandp,
                tc.tile_pool(name="expp", bufs=6) as expp,
                tc.tile_pool(name="prp", bufs=4) as prp,
                tc.tile_pool(name="ptp", bufs=2) as ptp,
                tc.tile_pool(name="smp", bufs=6) as smp,
                tc.tile_pool(name="outp", bufs=3) as outp,
                tc.tile_pool(name="pp_pos", bufs=3, space="PSUM") as pp_pos,
                tc.tile_pool(name="pp_tail", bufs=1, space="PSUM") as pp_tail,
                tc.tile_pool(name="pp_sc", bufs=2, space="PSUM") as pp_sc,
                tc.tile_pool(name="pp_tp", bufs=1, space="PSUM") as pp_tp,
                tc.tile_pool(name="pp_pv", bufs=1, space="PSUM") as pp_pv,
            ):
                # Band tiles interleave the DMA'd band (slot i) with a
                # constant [I I I I] pattern (slot 1-i) so one fp8 DoubleRow
                # matmul per 128x128 block injects s1 + s2^T:
                #   psum += lhsT[:,0].T@rhs[:,0] + lhsT[:,1].T@rhs[:,1]
                #         = I.T@s1 + s2.T@I
                s1c = bandp.tile([P, NB, 2, 4, S], f8, tag="s1c")
                s2c = bandp.tile([P, NB, 2, 4, S], f8, tag="s2c")
                i4 = bandp.tile([P, 4, S], f8, tag="i4")

                def build_bands_const():
                    nc.gpsimd.dma_start(i4[:, 0, 0:P], ident8[:])
                    nc.gpsimd.dma_start(i4[:, 0, P:2 * P], i4[:, 0, 0:P])
                    nc.gpsimd.dma_start(i4[:, 0, 2 * P:S], i4[:, 0, 0:2 * P])
                    nc.gpsimd.dma_start(i4[:, 1:2, :], i4[:, 0:1, :])
                    nc.gpsimd.dma_start(i4[:, 2:4, :], i4[:, 0:2, :])
                    for n in range(NB):
                        nc.gpsimd.dma_start(s1c[:, n, 1, :, :], i4[:])
                        nc.gpsimd.dma_start(s2c[:, n, 0, :, :], i4[:])

                def heads_of(b, h):
                    base = 64 * (h % 2)
                    ho = h // 2
                    return (
                        qT[base:base + DH, ho, b * S:(b + 1) * S],
                        kT[base:base + DH, ho, b * S:(b + 1) * S],
                        base,
                    )

                def heads8_of(b, h):
                    base = 64 * (h % 2)
                    ho = h // 2
                    return (
                        q8[base:base + DH, ho, b * S:(b + 1) * S],
                        k8[base:base + DH, ho, b * S:(b + 1) * S],
                        base,
                    )

                # W chunks loaded on demand (dout slice do), prefetched one
                # chunk ahead so the in-order PE queue never waits on them.
                w_r_q = Ws[0][:].rearrange("(o p) n -> p o n", p=P)
                w_r_k = Ws[1][:].rearrange("(o p) n -> p o n", p=P)
                w_tiles = {}

                def load_w_chunk(do):
                    for tag, w_r_ in (("wq", w_r_q), ("wk", w_r_k)):
                        wt = wp.tile([P, KO, P], bf16, tag=tag)
                        nc.gpsimd.dma_start(
                            wt[:], w_r_[:, :, do * P:(do + 1) * P]
                        )
                        w_tiles[(tag, do)] = wt

                load_w_chunk(0)

                def emit_qk_piece(do, wi, tb):
                    # one quarter of a projection chunk: interleaved between
                    # pairs so attn matmuls never queue behind a full chunk
                    if wi == 0 and tb == 0 and do + 1 < KO:
                        load_w_chunk(do + 1)
                    tag, dst, dst8, bias = (
                        ("wq", qT, q8, bq_sb) if wi == 0
                        else ("wk", kT, k8, bk_sb)
                    )
                    w_sb = w_tiles[(tag, do)]
                    ps = pp_pos.tile([P, 512], f32, tag="pos")
                    for kk in range(KO):
                        nc.tensor.matmul(
                            ps[:],
                            lhsT=w_sb[:, kk, :],
                            rhs=hsT[:, kk, tb * 512:(tb + 1) * 512],
                            start=(kk == 0),
                            stop=(kk == KO - 1),
                        )
                    nc.vector.tensor_scalar_add(
                        dst[:, do, tb * 512:(tb + 1) * 512],
                        ps[:], bias[:, do:do + 1],
                    )
                    nc.gpsimd.tensor_copy(
                        dst8[:, do, tb * 512:(tb + 1) * 512],
                        dst[:, do, tb * 512:(tb + 1) * 512],
                    )
                    if wi == 1 and tb == TB - 1:
                        del w_tiles[(tag, do)]
                        del w_tiles[("wq", do)]

                def emit_pos(b, h, n):
                    qh8, kh8, base = heads8_of(b, h)
                    scr = []
                    for side, (src, ew) in enumerate(((qh8, ERT8), (kh8, ET8))):
                        pool_ = posb if side == 0 else posbB
                        sb = pool_.tile([P, 4, JW], f8)
                        tail = pp_tail.tile([P, 4, P], f32, tag="tail")
                        for c in range(4):
                            jst = 384 - c * 128
                            lhs2 = src[:, c * P:(c + 1) * P].unsqueeze(
                                1).broadcast_to([DH, 2, P])
                            pp = pp_pos.tile([P, 512], f32, tag="pos")
                            nc.tensor.matmul(
                                pp[:],
                                lhsT=lhs2,
                                rhs=ew[base:base + DH, :, jst:jst + 512],
                                perf_mode=mybir.MatmulPerfMode.DoubleRow,
                                start=True, stop=True,
                            )
                            nc.tensor.matmul(
                                tail[:, c, :],
                                lhsT=lhs2,
                                rhs=ew[base:base + DH, :, jst + 512:jst + JW],
                                perf_mode=mybir.MatmulPerfMode.DoubleRow,
                                start=True, stop=True,
                            )
                            if c % 2 == 0:
                                nc.scalar.copy(sb[:, c, 0:512], pp[:])
                            else:
                                nc.vector.tensor_copy(sb[:, c, 0:512], pp[:])
                        if side == 0:
                            nc.scalar.copy(sb[:, :, 512:JW], tail[:])
                        else:
                            nc.vector.tensor_copy(sb[:, :, 512:JW], tail[:])
                        dpool = ddr if side == 0 else ddrB
                        dt_ = dpool.tile([4, P, JW], f8)
                        nc.sync.dma_start(
                            dt_[:].rearrange("c p w -> p c w"), sb[:]
                        )
                        scr.append(dt_)

                    # issue the band reads right away so they land during the
                    # previous pair's compute
                    nc.sync.dma_start(s1c[:, n, 0, :, :], _skew3(scr[0]))
                    nc.sync.dma_start(s2c[:, n, 1, :, :], _skew3(scr[1]))

                vw_tiles = {}

                def emit_v_gate(half, tos=None):
                    # v weights staged in dout halves to bound SBUF; each
                    # call fills vN[:, tos, half*512:(half+1)*512] (heads
                    # 8*half onward), which unblocks before any pair needs
                    # them.
                    if half == 0:
                        build_gw()
                    if half not in vw_tiles:
                        w_sb = vwp.tile([P, KO, 512], bf16, tag="wv")
                        vw_tiles[half] = w_sb
                        w_r = Ws[2][:].rearrange("(o p) n -> p o n", p=P)
                        for kk in range(0, KO, 2):
                            nc.gpsimd.dma_start(
                                w_sb[:, kk:kk + 2, :],
                                w_r[:, kk:kk + 2,
                                    half * 512:(half + 1) * 512],
                            )
                    w_sb = vw_tiles[half]
                    for to in (range(TP) if tos is None else tos):
                        ps = pp_sc.tile([P, S], f32, tag="ps")
                        for kk in range(KO):
                            nc.tensor.matmul(
                                ps[:],
                                lhsT=hsT[:, kk, to * P:(to + 1) * P],
                                rhs=w_sb[:, kk, :],
                                start=(kk == 0),
                                stop=False,
                            )
                        nc.tensor.matmul(
                            ps[:], lhsT=ones_row[:],
                            rhs=bv_row[0:1, half * 512:(half + 1) * 512],
                            start=False, stop=True,
                        )
                        # fold the clipped-softmax scale c into v
                        nc.vector.tensor_scalar_mul(
                            vN[:, to, half * 512:(half + 1) * 512],
                            ps[:], CSCALE,
                        )
                    if half == 1:
                        return
                    for to in (range(TP) if tos is None else tos):
                        psg = pp_pv.tile([P, DH], f32, tag="pv")
                        for kk in range(KO):
                            nc.tensor.matmul(
                                psg[:, 0:H],
                                lhsT=hsT[:, kk, to * P:(to + 1) * P],
                                rhs=gw_sb[:, kk, :],
                                start=(kk == 0),
                                stop=False,
                            )
                        nc.tensor.matmul(
                            psg[:, 0:H], lhsT=ones_row[:], rhs=gb_row[:],
                            start=False, stop=True,
                        )
                        nc.scalar.activation(gateT[:, to, :], psg[:, 0:H], AF.Sigmoid)

                def emit_attn(b, h, n):
                    qh, kh, base = heads_of(b, h)
                    sums = smp.tile([P, 4], f32, tag="sums")
                    exps = []
                    for lc in range(4):
                        ps = pp_sc.tile([P, S], f32, tag="ps")
                        nc.tensor.matmul(
                            ps[:],
                            lhsT=qh[:, lc * P:(lc + 1) * P],
                            rhs=kh[:],
                            start=True, stop=False,
                        )
                        for rc in range(4):
                            nc.tensor.matmul(
                                ps[:, rc * P:(rc + 1) * P],
                                lhsT=s2c[:, n, :, rc, lc * P:(lc + 1) * P],
                                rhs=s1c[:, n, :, lc, rc * P:(rc + 1) * P],
                                perf_mode=mybir.MatmulPerfMode.DoubleRow,
                                start=False, stop=(rc == 3),
                            )
                        ex = expp.tile([P, S], bf16)
                        nc.scalar.activation(
                            ex[:], ps[:], AF.Exp, scale=0.125,
                            accum_out=sums[:, lc:lc + 1],
                        )
                        exps.append(ex)

                    # probs = c*softmax + gamma clipped to [0,1]
                    #       = (c/sums) * max(ex - th, 0),  th = |gamma|*sums/c
                    # c is folded into vN, (1/sums)*gate into the ctx scale.
                    nth = smp.tile([P, 4], f32, tag="nth")
                    nc.vector.tensor_scalar_mul(nth[:], sums[:], -GOC)
                    inv = smp.tile([P, 4], f32, tag="inv")
                    nc.vector.reciprocal(inv[:], sums[:])
                    gs = smp.tile([P, 4], f32, tag="gs")
                    nc.vector.tensor_tensor(
                        gs[:], inv[:],
                        gateT[:, b * 4:b * 4 + 4, h:h + 1]
                        .rearrange("p a o -> p (a o)"),
                        mybir.AluOpType.mult,
                    )

                    # probs (bf16) -> probsT via PE transpose; one batched
                    # psum->sbuf copy per l-chunk
                    pT = ptp.tile([P, 4, S], bf16)
                    for lc in range(4):
                        pr = prp.tile([P, S], bf16)
                        reng = nc.vector if lc == 0 else nc.gpsimd
                        reng.tensor_scalar(
                            pr[:], exps[lc][:], nth[:, lc:lc + 1], 0.0,
                            op0=mybir.AluOpType.add,
                            op1=mybir.AluOpType.max,
                        )
                        tp = pp_tp.tile([P, 4, P], bf16)
                        for rc in range(4):
                            nc.tensor.transpose(
                                tp[:, rc, :], pr[:, rc * P:(rc + 1) * P],
                                ident_bf[:],
                            )
                        nc.vector.tensor_copy(
                            pT[:, :, lc * P:(lc + 1) * P], tp[:])

                    # ctx = probs @ v, gated; one batched out DMA per pair
                    ot = outp.tile([P, 4, DH], f32)
                    pv = pp_pv.tile([P, 4, DH], f32, tag="pv")
                    for lc in range(4):
                        for rc in range(4):
                            nc.tensor.matmul(
                                pv[:, lc, :],
                                lhsT=pT[:, rc, lc * P:(lc + 1) * P],
                                rhs=vN[:, b * 4 + rc, h * DH:(h + 1) * DH],
                                start=(rc == 0), stop=(rc == 3),
                            )
                    nc.vector.tensor_tensor(
                        ot[:], pv[:],
                        gs[:].unsqueeze(2).broadcast_to([P, 4, DH]),
                        mybir.AluOpType.mult,
                    )
                    nc.gpsimd.dma_start(
                        out[b * S:(b + 1) * S, h * DH:(h + 1) * DH]
                        .rearrange("(c p) d -> p c d", p=P),
                        ot[:],
                    )

                from collections import deque
                pending = deque()
                DEPTH = 3
                def pieces(do):
                    return [("qkp", (do, wi, tb))
                            for wi in range(2) for tb in range(TB)]

                seq = list(pieces(0))
                for do in range(KO):
                    prs = [("pair", (b, 2 * do + dh))
                           for dh in range(2) for b in range(BL)]
                    if do == 0:
                        # v/gate dout-half 0 is needed by the first popped
                        # attn (i=3); emit it after 3 pos stages so the
                        # softmax engines start immediately.
                        prs = prs[:3] + [("vg", (0, None))] + prs[3:]
                    nxt = pieces(do + 1) if do + 1 < KO else []
                    if do == 2:
                        nxt = [x for pr in zip(
                            nxt, [("vg", (1, [t])) for t in range(4)])
                            for x in pr]
                    if do == 3:
                        nxt = [x for pr in zip(
                            nxt, [("vg", (1, [t])) for t in range(4, 8)])
                            for x in pr]
                    import itertools
                    merged = [x for pair_ in itertools.zip_longest(prs, nxt)
                              for x in pair_ if x is not None]
                    seq += merged
                i = 0
                for kind, arg in seq:
                    if kind == "qkp":
                        emit_qk_piece(*arg)
                    elif kind == "vg":
                        emit_v_gate(*arg)
                    else:
                        b, h = arg
                        emit_pos(b, h, i % NB)
                        if i == 0:
                            build_bands_const()
                        pending.append((b, h, i % NB))
                        if len(pending) > DEPTH:
                            emit_attn(*pending.popleft())
                        i += 1
                while pending:
                    emit_attn(*pending.popleft())


_NC_CACHE = {}


def _get_program():
    if "nc" not in _NC_CACHE:
        _NC_CACHE["nc"] = build_program()
    return _NC_CACHE["nc"]


def make_in_maps(inputs):
    hs = np.ascontiguousarray(np.asarray(inputs["hidden_states"], dtype=np.float32))
    maps = []
    shared = {
        "Wq": np.asarray(inputs["Wq"], np.float32),
        "Wk": np.asarray(inputs["Wk"], np.float32),
        "Wv": np.asarray(inputs["Wv"], np.float32),
        "bq": np.asarray(inputs["bq"], np.float32),
        "bk": np.asarray(inputs["bk"], np.float32),
        "bv": np.asarray(inputs["bv"], np.float32),
        "emb": np.asarray(inputs["dist_emb"], np.float32),
        "embr": np.ascontiguousarray(
            np.asarray(inputs["dist_emb"], np.float32)[::-1]
        ),
        "gw": np.asarray(inputs["gate_w"], np.float32),
        "gb": np.asarray(inputs["gate_b"], np.float32),
    }
    for c in range(NCORES):
        m = dict(shared)
        m["hs"] = np.ascontiguousarray(
            hs[c * BL:(c + 1) * BL].reshape(TOK, D)
        )
        maps.append(m)
    return maps


def kernel(**inputs):
    from concourse.bass_utils import run_bass_kernel_spmd

    nc = _get_program()
    in_maps = make_in_maps(inputs)
    res = run_bass_kernel_spmd(nc, in_maps, core_ids=list(range(NCORES)))
    return np.concatenate(
        [res.results[c]["out"].reshape(BL, S, D) for c in range(NCORES)], axis=0
    )

